# revision 4
# baseline (speedup 1.0000x reference)
"""Compressed-KV GPT-2 attention block on 8 TRN2 NeuronCores — v2.

Sharding: batch x head-group. Core c: batch b = c//4, heads 4*(c%4)..+4
(= 2 head-pairs). Transposed-activation layout ([dim, seq] on partitions);
each core emits a partial c_proj output^T; host sums 4 partials per batch.

v2 structural changes over the 264us baseline:
  - rank-32 factorization: k' = (x @ (w_k wk_c)) @ (wk_d/sqrt(hd)) and
    v_dec = (x @ (w_v wv_c)) @ wv_d, so the qkv matmul emits 32-wide
    compressed k/v columns (512 total vs 768) and the decompress runs as
    tiny reused-stationary matmuls (k': K=32 row-tiled 2x2; v: one
    block-diagonal rhs decompresses+transposes both heads per key chunk).
  - head-pair row-tiling: even head on array rows 0-63, odd head on rows
    64-127 (implicit tile_position from base partitions), so the K=64
    score matmuls for a pair run concurrently in the PE array.
  - paired PSUM tiles [128,1024] (2 banks): qkv/score/proj matmuls fill 2
    banks that drain with ONE wide DVE/ACT op, halving per-instruction
    overhead (exp esp.: ACTIVATE pays a 352-cycle fixed cost).
  - softmax normalize: DVE reciprocal of the PSUM den row -> DRAM bounce
    -> broadcast DMA -> one DVE multiply (replaces the 4-DMA reshape
    chain per iteration); rec-path DMAs ride the idle GpSimd queue.
  - c_proj for seq-block sb is emitted inside attention of sb+1 so its
    matmuls/stores overlap; only the last block's projection is a tail.
"""

import sys

if "/opt/trn_rl_repo" not in sys.path:
    sys.path.insert(0, "/opt/trn_rl_repo")

import numpy as np
import ml_dtypes

BF16 = ml_dtypes.bfloat16

B, S, D = 2, 2048, 1024
H, hd, C = 16, 64, 32
NCORES = 8
SB = 512
NSB = S // SB      # 4 seq blocks of 512
NKT = S // 128     # 16 key tiles of 128
DC = D // 128      # 8 contraction chunks for qkv

_cache = {}


def _build():
    import concourse.bacc as bacc
    import concourse.tile as tile
    import concourse.mybir as mybir

    dt = mybir.dt
    f32, bf16 = dt.float32, dt.bfloat16
    Exp = mybir.ActivationFunctionType.Exp
    mult = mybir.AluOpType.mult

    nc = bacc.Bacc("TRN2", target_bir_lowering=False, debug=False, num_devices=NCORES)

    hidden_t = nc.dram_tensor("hidden_t", [D, S], bf16, kind="ExternalInput")
    w_qkv = nc.dram_tensor("w_qkv", [D, 512], bf16, kind="ExternalInput")
    b_qkv = nc.dram_tensor("b_qkv", [128, 4], f32, kind="ExternalInput")
    wkd2_d = nc.dram_tensor("wkd2", [2, 64, 128], bf16, kind="ExternalInput")
    wvd2_d = nc.dram_tensor("wvd2", [2, 64, 128], bf16, kind="ExternalInput")
    w_proj = nc.dram_tensor("w_proj", [2, 128, D], bf16, kind="ExternalInput")
    maskd = nc.dram_tensor("maskd", [128, 128], bf16, kind="ExternalInput")
    out_t = nc.dram_tensor("out_t", [D, S], bf16, kind="ExternalOutput")

    with tile.TileContext(nc) as tc:
        with (
            tc.tile_pool(name="persist", bufs=1) as pp,
            tc.tile_pool(name="work", bufs=2) as wp,
            tc.tile_pool(name="epool", bufs=6) as ep,
            tc.tile_pool(name="dscr", bufs=4, space="DRAM") as dr,
            tc.tile_pool(name="psb", bufs=2, space="PSUM") as psb,
            tc.tile_pool(name="pss", bufs=4, space="PSUM") as pss,
        ):
            # ---- loads: qkv weights + hidden first (consumption order) ----
            bias = pp.tile([128, 4], f32, name="bias")
            nc.sync.dma_start(bias[:], b_qkv.ap())
            wq = []
            for d in range(DC):
                w = pp.tile([128, 512], bf16, name=f"wq{d}")
                nc.sync.dma_start(w[:], w_qkv.ap()[d * 128:(d + 1) * 128, :])
                wq.append(w)
            hT = [pp.tile([128, S], bf16, name=f"hT{d}") for d in range(DC)]
            for sbp in range(2):
                for d in range(DC):
                    eng = nc.sync if d % 2 == 0 else nc.gpsimd
                    eng.dma_start(
                        hT[d][:, sbp * 1024:(sbp + 1) * 1024],
                        hidden_t.ap()[d * 128:(d + 1) * 128, sbp * 1024:(sbp + 1) * 1024],
                    )
                if sbp == 0:
                    maskt = pp.tile([128, 128], bf16, name="maskt")
                    nc.gpsimd.dma_start(maskt[:], maskd.ap())
                    wkd2, wvd2, wpj = [], [], []
                    for p in range(2):
                        t = pp.tile([128, 128], bf16, name=f"wkd2_{p}")
                        nc.gpsimd.dma_start(t[0:64, :], wkd2_d.ap()[p])
                        wkd2.append(t)
                        t2 = pp.tile([128, 128], bf16, name=f"wvd2_{p}")
                        nc.gpsimd.dma_start(t2[64:128, :], wvd2_d.ap()[p])
                        wvd2.append(t2)
                        t3 = pp.tile([128, D], bf16, name=f"wpj{p}")
                        nc.gpsimd.dma_start(t3[:], w_proj.ap()[p])
                        wpj.append(t3)

            # ---- qkv^T: m-blocks [q0|q1],[q2|q3],[kc0|kc1|vc0|vc1]x2 ----
            qq = [pp.tile([128, S], bf16, name=f"qq{p}") for p in range(2)]
            KC = [pp.tile([128, S], bf16, name=f"kc{p}") for p in range(2)]
            dests = qq + KC
            for sbp in range(2):
                for mb in range(4):
                    ps = psb.tile([128, 1024], f32, tag="ps2", name="psq")
                    for j in range(2):
                        sb = 2 * sbp + j
                        for d in range(DC):
                            nc.tensor.matmul(
                                ps[:, j * 512:(j + 1) * 512],
                                wq[d][:, mb * 128:(mb + 1) * 128],
                                hT[d][:, sb * 512:(sb + 1) * 512],
                                start=(d == 0),
                                stop=(d == DC - 1),
                            )
                    nc.vector.tensor_scalar_add(
                        out=dests[mb][:, sbp * 1024:(sbp + 1) * 1024],
                        in0=ps[:],
                        scalar1=bias[:, mb:mb + 1],
                    )

            # ---- decompress: k' (K=32 row-tiled) + v (block-diag rhs) ----
            kk = [pp.tile([128, S], bf16, name=f"kk{p}") for p in range(2)]
            vdo = [pp.tile([128, NKT * 130], bf16, name=f"vdo{p}") for p in range(2)]
            for p in range(2):
                nc.vector.memset(vdo[p][:], 1.0)
            for sb in range(NSB):
                sl = slice(sb * SB, (sb + 1) * SB)
                for p in range(2):
                    psK = pss.tile([128, 512], f32, tag="ps1", name="psK")
                    nc.tensor.matmul(
                        psK[0:64, :], wkd2[p][0:32, 0:64], KC[p][0:32, sl]
                    )
                    nc.tensor.matmul(
                        psK[64:128, :], wkd2[p][32:64, 64:128], KC[p][32:64, sl]
                    )
                    nc.vector.tensor_copy(kk[p][:, sl], psK[:])
                for p in range(2):
                    psC = pss.tile([128, 512], f32, tag="ps1", name="psC")
                    for cch in range(4):
                        st = 4 * sb + cch
                        nc.tensor.matmul(
                            psC[:, cch * 128:(cch + 1) * 128],
                            KC[p][64:128, st * 128:(st + 1) * 128],
                            wvd2[p][64:128, :],
                        )
                    src = psC[:].rearrange("p (c w) -> p c w", w=128)
                    dst = vdo[p][:, 4 * sb * 130:(4 * sb + 4) * 130].rearrange(
                        "p (c w) -> p c w", w=130
                    )
                    nc.vector.tensor_copy(dst[:, :, 0:64], src[:, :, 0:64])
                    nc.vector.tensor_copy(dst[:, :, 65:129], src[:, :, 64:128])

            # ---- attention (qsb-outer so c_proj overlaps) + merge ----
            attn = [pp.tile([128, S], bf16, name=f"attn{p}") for p in range(2)]

            def emit_proj(sb, last=False):
                sl = slice(sb * SB, (sb + 1) * SB)
                for mbp in range(4):
                    psP = psb.tile([128, 1024], f32, tag="ps2", name="psP")
                    for j in range(2):
                        mb = 2 * mbp + j
                        for p in range(2):
                            nc.tensor.matmul(
                                psP[:, j * 512:(j + 1) * 512],
                                wpj[p][:, mb * 128:(mb + 1) * 128],
                                attn[p][:, sl],
                                start=(p == 0),
                                stop=(p == 1),
                            )
                    stage = wp.tile([128, 1024], bf16, tag="stage", bufs=3, name="stage")
                    if last:
                        nc.scalar.activation(
                            stage[:, 0:512], psP[:, 0:512],
                            mybir.ActivationFunctionType.Copy,
                        )
                        nc.vector.tensor_copy(stage[:, 512:1024], psP[:, 512:1024])
                    else:
                        nc.vector.tensor_copy(stage[:], psP[:])
                    for j in range(2):
                        mb = 2 * mbp + j
                        nc.sync.dma_start(
                            out_t.ap()[mb * 128:(mb + 1) * 128, sl],
                            stage[:, j * 512:(j + 1) * 512],
                        )

            for qsb in range(NSB):
                qsl = slice(qsb * SB, (qsb + 1) * SB)
                nkb = 4 * qsb + 4
                for hp in range(2):
                    pso_e = pss.tile([128, 512], f32, tag="ps1", name="psoE")
                    pso_o = pss.tile([128, 512], f32, tag="ps1", name="psoO")

                    def emit_attnv(e2_e, e2_o, kbA, kbB, c0A, c0B):
                        for (e2, pso, oh) in ((e2_e, pso_e, 0), (e2_o, pso_o, 65)):
                            for (jj, kb, c0) in ((0, kbA, c0A), (1, kbB, c0B)):
                                nc.tensor.matmul(
                                    pso[0:65, c0:512],
                                    vdo[hp][:, kb * 130 + oh:kb * 130 + oh + 65],
                                    e2[:, jj * 512 + c0:(jj + 1) * 512],
                                    start=(kb == 0),
                                    stop=(kb == nkb - 1),
                                )

                    prev = None
                    for kbp in range(nkb // 2):
                        kbA, kbB = 2 * kbp, 2 * kbp + 1
                        rA, rB = kbA - 4 * qsb, kbB - 4 * qsb
                        c0A, c0B = max(rA, 0) * 128, max(rB, 0) * 128
                        psS_e = psb.tile([128, 1024], f32, tag="ps2", name="psSe")
                        psS_o = psb.tile([128, 1024], f32, tag="ps2", name="psSo")
                        for (jj, kb, c0) in ((0, kbA, c0A), (1, kbB, c0B)):
                            ksl = slice(kb * 128, (kb + 1) * 128)
                            qs2 = slice(qsb * SB + c0, (qsb + 1) * SB)
                            nc.tensor.matmul(
                                psS_e[:, jj * 512 + c0:(jj + 1) * 512],
                                kk[hp][0:64, ksl], qq[hp][0:64, qs2],
                            )
                            nc.tensor.matmul(
                                psS_o[:, jj * 512 + c0:(jj + 1) * 512],
                                kk[hp][64:128, ksl], qq[hp][64:128, qs2],
                            )
                        e2_e = ep.tile([128, 1024], bf16, tag="e2", name="e2e")
                        e2_o = ep.tile([128, 1024], bf16, tag="e2", name="e2o")
                        band = rA >= 0
                        for (e2, psS) in ((e2_e, psS_e), (e2_o, psS_o)):
                            if not band:
                                nc.scalar.activation(e2[:], psS[:], Exp)
                            else:
                                nc.scalar.activation(
                                    e2[:, c0A:512], psS[:, c0A:512], Exp
                                )
                                nc.scalar.activation(
                                    e2[:, 512 + c0B:1024], psS[:, 512 + c0B:1024], Exp
                                )
                                for lo in (c0A, 512 + c0B):
                                    nc.vector.tensor_tensor(
                                        e2[:, lo:lo + 128], e2[:, lo:lo + 128],
                                        maskt[:], mult,
                                    )
                        if prev is not None:
                            emit_attnv(*prev)
                        prev = (e2_e, e2_o, kbA, kbB, c0A, c0B)
                    emit_attnv(*prev)

                    # normalize: num/den; den is pso row 64 (ones col of vdo)
                    for (pso, even) in ((pso_e, True), (pso_o, False)):
                        rec = wp.tile([128, 512], bf16, tag="rec", name="rec")
                        with nc.allow_low_precision(reason="softmax denom recip bf16"):
                            nc.vector.reciprocal(rec[64:65, :], pso[64:65, :])
                        recd = dr.tile([512], bf16, tag="recd", name="recd")
                        nc.gpsimd.dma_start(recd[:], rec[64:65, :])
                        bc = wp.tile([64, 512], bf16, tag="bc", name="bc")
                        nc.gpsimd.dma_start(
                            bc[:], recd[:].unsqueeze(0).to_broadcast([64, 512])
                        )
                        if even:
                            nc.vector.tensor_tensor(
                                attn[hp][0:64, qsl], pso[0:64, :], bc[:], mult
                            )
                        else:
                            atmp = wp.tile([64, 512], bf16, tag="atmp", name="atmp")
                            nc.vector.tensor_tensor(
                                atmp[:], pso[0:64, :], bc[:], mult
                            )
                            nc.gpsimd.dma_start(attn[hp][64:128, qsl], atmp[:])

                    if hp == 0 and qsb > 0:
                        emit_proj(qsb - 1)
            emit_proj(NSB - 1, last=True)

    nc.compile()
    return nc


def _prep_inputs(hidden_states, w_attn, b_attn, wk_c, wv_c, wk_d, wv_d, w_proj):
    """Per-core input maps (host-side shard + rank-32 fold + bf16 cast).

    k' = k @ (wk_c wk_d / sqrt(hd)) factors as (x @ (w_k wk_c)) @ (wk_d/8):
    the 32-wide compressed projections fold into w_qkv columns, the 32->64
    decompressors ship as tiny per-pair matrices. Same for v with wv_*.
    """
    f64 = np.float64
    hidden_T = [np.ascontiguousarray(hidden_states[b].T).astype(BF16) for b in range(B)]
    wq_h = lambda h: w_attn[:, h * hd:(h + 1) * hd]
    wkcf = lambda h: (w_attn[:, D + h * hd:D + (h + 1) * hd].astype(f64)
                      @ wk_c[h].astype(f64)).astype(np.float32)
    wvcf = lambda h: (w_attn[:, 2 * D + h * hd:2 * D + (h + 1) * hd].astype(f64)
                      @ wv_c[h].astype(f64)).astype(np.float32)
    bq_h = lambda h: b_attn[h * hd:(h + 1) * hd]
    bkc = lambda h: (b_attn[D + h * hd:D + (h + 1) * hd].astype(f64)
                     @ wk_c[h].astype(f64)).astype(np.float32)
    bvc = lambda h: (b_attn[2 * D + h * hd:2 * D + (h + 1) * hd].astype(f64)
                     @ wv_c[h].astype(f64)).astype(np.float32)
    kk_ = np.arange(128).reshape(128, 1)
    cg = np.arange(128).reshape(1, 128)
    mask = np.ascontiguousarray((kk_ <= cg).astype(BF16))
    in_maps = []
    for c in range(NCORES):
        b = c // 4
        hs4 = [4 * (c % 4) + i for i in range(4)]
        pairs = [(hs4[0], hs4[1]), (hs4[2], hs4[3])]
        wcols, bcols = [], []
        for (he, ho) in pairs:
            wcols.append(np.concatenate([wq_h(he), wq_h(ho)], 1))
            bcols.append(np.concatenate([bq_h(he), bq_h(ho)]))
        for (he, ho) in pairs:
            wcols.append(np.concatenate([wkcf(he), wkcf(ho), wvcf(he), wvcf(ho)], 1))
            bcols.append(np.concatenate([bkc(he), bkc(ho), bvc(he), bvc(ho)]))
        w_qkv_l = np.concatenate(wcols, 1).astype(BF16)          # [1024, 512]
        b_qkv_l = np.stack(bcols, 1).astype(np.float32)          # [128, 4]
        wkd2 = np.zeros((2, 64, 128), np.float32)
        wvd2 = np.zeros((2, 64, 128), np.float32)
        for p, (he, ho) in enumerate(pairs):
            wkd2[p, 0:32, 0:64] = wk_d[he] / np.sqrt(hd)
            wkd2[p, 32:64, 64:128] = wk_d[ho] / np.sqrt(hd)
            wvd2[p, 0:32, 0:64] = wv_d[he]
            wvd2[p, 32:64, 64:128] = wv_d[ho]
        wpj_l = np.stack([
            np.concatenate([w_proj[he * hd:(he + 1) * hd, :],
                            w_proj[ho * hd:(ho + 1) * hd, :]], 0)
            for (he, ho) in pairs
        ])
        in_maps.append({
            "hidden_t": hidden_T[b],
            "w_qkv": w_qkv_l,
            "b_qkv": b_qkv_l,
            "wkd2": wkd2.astype(BF16),
            "wvd2": wvd2.astype(BF16),
            "w_proj": wpj_l.astype(BF16),
            "maskd": mask,
        })
    return in_maps


def kernel(
    hidden_states,
    w_attn,
    b_attn,
    w_proj,
    b_proj,
    wk_c,
    wv_c,
    wk_d,
    wv_d,
    _trace=False,
):
    from concourse.bass_utils import run_bass_kernel_spmd

    if "nc" not in _cache:
        _cache["nc"] = _build()
    nc = _cache["nc"]

    in_maps = _prep_inputs(
        np.asarray(hidden_states),
        np.asarray(w_attn),
        np.asarray(b_attn),
        np.asarray(wk_c),
        np.asarray(wv_c),
        np.asarray(wk_d),
        np.asarray(wv_d),
        np.asarray(w_proj),
    )
    res = run_bass_kernel_spmd(
        nc, in_maps, core_ids=list(range(NCORES)), trace=_trace
    )
    out = np.empty((B, S, D), np.float32)
    for b in range(B):
        acc = np.zeros((D, S), np.float32)
        for c in range(4 * b, 4 * b + 4):
            acc += res.results[c]["out_t"].astype(np.float32)
        out[b] = acc.T + np.asarray(b_proj, np.float32)
    if _trace:
        _cache["last_exec_time_ns"] = res.exec_time_ns
        _cache["last_results"] = res
    return out
